# revision 10
# baseline (speedup 1.0000x reference)
"""CliffordLinearSimple on 8 Trainium2 NeuronCores.

Math (per reference):
    sv   = x[:, :, SV_IDX]                      # [B, IN_F, 9]  (scalar+vector slots)
    svo  = sv.reshape(B, IN_F*9) @ W.T + b      # [B, OUT_F*9]
    v    = svo.reshape(B, OUT_F, 9)[:, :, 1:]   # [B, OUT_F, 8]
    biv  = v[:, :, IU] * v[:, :, JU]            # [B, OUT_F, 28]
    out[..., SV_IDX] = svo; out[..., BIV_IDX] = biv; rest 0

Distribution: tensor-parallel over OUT_F (row-split W): core c owns out
slots [c*1152, (c+1)*1152).  The device does ONLY the GEMM
C[256, 1152] = svT.T @ W_c in bf16 (fp32 PSUM) and writes C back as
bf16; bias add, the 28 bivector products, and the scatter into the
[256, 1024, 256] multivector output all happen on the host in fp32
(exact, and bias in f32 is more accurate than the old bf16 device-side
bias matmul).

Schedule: k-outer over 18 groups of 4 k-tiles (K = 9216 = 72*128).
All six PSUM accumulators (2 batch tiles x 3 out-column tiles of
504+504+144) live for the whole kernel, so every group's DMA feeds the
same fixed compute:DMA ratio -- no front-loaded phase.  Three DMA
queues stream inputs concurrently (the old kernel's two HWDGE rings
capped input at ~334 GB/s while SWDGE sat idle until the output
drain): sync ring carries the n0 W chunks, scalar ring the n1 chunks,
and gpsimd/SWDGE carries svT + the narrow n2 chunks.  W is packed on
the host per (ring, group) into flat DRAM blocks so every W DMA is one
fully-sequential read; svT chunks are per-partition-contiguous slices.

Tail is just the final PSUM->SBUF bf16 casts (DVE) plus six ~100KB
output DMAs, instead of the old ~14us f32 compact-output + bivector
drain.  16 junk warm-up matmuls at the top keep the PE p-state ramp
off the critical path.
"""
import sys

if "/opt/trn_rl_repo" not in sys.path:
    sys.path.insert(0, "/opt/trn_rl_repo")

from contextlib import ExitStack

import ml_dtypes
import numpy as np

import concourse.bass as bass
import concourse.tile as tile
from concourse import bacc, mybir
from concourse.bass_utils import run_bass_kernel_spmd

ALG_DIM = 8
D1 = 9
MV_DIM = 256
B, IN_F, OUT_F = 256, 1024, 1024
POW2 = np.array([2 ** i for i in range(ALG_DIM)])
SV_IDX = np.concatenate([[0], POW2])
IU, JU = np.triu_indices(ALG_DIM, 1)
BIV_IDX = POW2[IU] + POW2[JU]
NCORES = 8
OF = OUT_F // NCORES          # 128 out features per core
N_CORE = OF * D1              # 1152 out slots per core
KT = IN_F * D1 // 128         # 72 k-tiles
BT = 2                        # batch tiles of 128

# out-column tiles (PSUM bank is 512 f32 wide) and k-groups.
# A DMA queue dispatches ~1 descriptor (= one partition line of
# G*NTILE*2 bytes) per ~35ns regardless of size, so queue throughput is
# line-size-bound: 4KB lines -> ~115 GB/s, 14KB -> ~260 GB/s ceiling.
# Small leading groups start the pipeline early (lines don't matter
# before the queues saturate); big trailing groups carry the bulk.
NTILES = (504, 504, 144)
NOFF = [sum(NTILES[:i]) for i in range(len(NTILES))]
KGRPS = [4, 6, 8, 12, 14, 14, 14]
KOFF = [sum(KGRPS[:i]) for i in range(len(KGRPS))]
NGRP = len(KGRPS)
# ring -> which n-tiles' W chunks it carries (rings: sync, scalar, gpsimd).
# The two HWDGE queues carry ONLY the wide n0/n1 W streams (pure big
# lines); SWDGE carries svT + the narrow n2 stream, both of which would
# otherwise poison the HWDGE queues' descriptor mix with small lines.
RING_N = ([0], [1], [2])
SVT_RING = 2
WARM = 16


def build_core_program():
    assert sum(KGRPS) == KT and sum(NTILES) == N_CORE
    f32, bf16 = mybir.dt.float32, mybir.dt.bfloat16

    nc = bacc.Bacc("TRN2", target_bir_lowering=False, debug=False)
    svT_d = nc.dram_tensor("svT", [128, KT, B], bf16, kind="ExternalInput").ap()
    W_ds = [
        nc.dram_tensor(
            f"Wr{r}", [128 * sum(KGRPS) * sum(NTILES[n] for n in ns)], bf16,
            kind="ExternalInput",
        ).ap()
        if ns else None
        for r, ns in enumerate(RING_N)
    ]
    out_d = nc.dram_tensor("outc", [B, N_CORE], bf16, kind="ExternalOutput").ap()

    with tile.TileContext(nc) as tc:
        with ExitStack() as ctx:
            const = ctx.enter_context(tc.tile_pool(name="const", bufs=1))
            wpools = [
                ctx.enter_context(tc.tile_pool(name=f"wp{r}", bufs=4))
                for r in range(len(RING_N))
            ]
            spool = ctx.enter_context(tc.tile_pool(name="spool", bufs=6))
            warmpool = ctx.enter_context(
                tc.tile_pool(name="warmpool", bufs=1, space="PSUM")
            )
            pspool = ctx.enter_context(
                tc.tile_pool(name="pspool", bufs=BT * len(NTILES), space="PSUM")
            )
            rings = [nc.sync, nc.scalar, nc.gpsimd]

            svT = const.tile([128, KT, B], bf16)

            # PE warm-up with no DMA deps: junk matmuls into a spare PSUM
            # bank release the HAM clock gate while the first W/svT chunks
            # are still in flight.
            warm_ps = warmpool.tile([128, 504], f32, name="warm", tag="warm")
            ones = const.tile([1, 128], bf16)
            nc.vector.memset(ones[:], 1.0)
            warm_rhs = const.tile([1, 504], bf16)
            nc.vector.memset(warm_rhs[:], 0.0)
            for _ in range(WARM):
                nc.tensor.matmul(
                    warm_ps[:], ones[:], warm_rhs[:],
                    start=True, stop=True, skip_group_check=True,
                )

            ps = {
                (m, n): pspool.tile([128, NTILES[n]], f32, name=f"ps{m}_{n}", tag="ps")
                for n in range(len(NTILES))
                for m in range(BT)
            }

            woff = [0] * len(RING_N)
            for g, G in enumerate(KGRPS):
                k0 = KOFF[g]
                # svT chunk g: per-partition contiguous 512*G-byte slice
                rings[SVT_RING].dma_start(svT[:, k0:k0 + G, :], svT_d[:, k0:k0 + G, :])
                wts = []
                for r, ns in enumerate(RING_N):
                    if not ns:
                        wts.append(None)
                        continue
                    cols = G * sum(NTILES[n] for n in ns)
                    wt = wpools[r].tile([128, cols], bf16, name=f"wt{r}", tag=f"wt{r}")
                    sz = 128 * cols
                    rings[r].dma_start(
                        wt[:],
                        W_ds[r][woff[r]:woff[r] + sz].rearrange("(p c) -> p c", p=128),
                    )
                    woff[r] += sz
                    wts.append(wt)
                for r, ns in enumerate(RING_N):
                    coff = 0
                    for n in ns:
                        nt = NTILES[n]
                        for m in range(BT):
                            for kl in range(G):
                                nc.tensor.matmul(
                                    ps[(m, n)][:],
                                    svT[:, k0 + kl, m * 128:(m + 1) * 128],
                                    wts[r][:, coff + kl * nt:coff + (kl + 1) * nt],
                                    start=(g == 0 and kl == 0),
                                    stop=(g == NGRP - 1 and kl == G - 1),
                                )
                        coff += G * nt

            # drain: cast each accumulator to bf16 and write it out on the
            # ring that just finished streaming that n-tile's W chunks
            for n in range(len(NTILES)):
                for m in range(BT):
                    st = spool.tile([128, NTILES[n]], bf16, name="st", tag="st")
                    nc.vector.tensor_copy(st[:], ps[(m, n)][:])
                    rings[n % 2].dma_start(
                        out_d[m * 128:(m + 1) * 128, NOFF[n]:NOFF[n] + NTILES[n]],
                        st[:],
                    )

    nc.finalize()
    return nc


_PROGRAM = None


def _get_program():
    global _PROGRAM
    if _PROGRAM is None:
        _PROGRAM = build_core_program()
    return _PROGRAM


def _prep_inputs(x, W, b):
    bf16 = ml_dtypes.bfloat16
    # svT[p, kt, m] = sv[m, kt*128 + p], sv = x[:, :, SV_IDX] flattened
    sv = np.ascontiguousarray(x[:, :, SV_IDX]).reshape(B, IN_F * D1)
    svT = np.ascontiguousarray(sv.reshape(B, KT, 128).transpose(2, 1, 0)).astype(bf16)

    Wb = W.astype(bf16).reshape(NCORES, N_CORE, KT * 128)
    in_maps = []
    for c in range(NCORES):
        m = {"svT": svT}
        for r, ns in enumerate(RING_N):
            if not ns:
                continue
            parts = []
            for g, G in enumerate(KGRPS):
                k0 = KOFF[g]
                for n in ns:
                    # [p, kl, col] block, raveled: matches the device-side
                    # [128, G*NTILE] tile slice layout
                    blk = Wb[c, NOFF[n]:NOFF[n] + NTILES[n],
                             k0 * 128:(k0 + G) * 128]              # [col, G*128]
                    blk = blk.reshape(NTILES[n], G, 128).transpose(2, 1, 0)
                    parts.append(np.ascontiguousarray(blk).ravel())
            m[f"Wr{r}"] = np.concatenate(parts)
        in_maps.append(m)
    return in_maps


def run(x, W, b, trace=False):
    x = np.asarray(x, dtype=np.float32)
    W = np.asarray(W, dtype=np.float32)
    b = np.asarray(b, dtype=np.float32)
    in_maps = _prep_inputs(x, W, b)
    nc = _get_program()
    res = None
    for attempt in range(3):
        try:
            res = run_bass_kernel_spmd(
                nc, in_maps, core_ids=list(range(NCORES)), trace=trace
            )
            break
        except Exception:
            if attempt == 2:
                raise
            import time as _time
            _time.sleep(5)
    # host-side epilogue in f32: bias, bivector products, scatter
    svo = np.concatenate(
        [np.asarray(res.results[c]["outc"]) for c in range(NCORES)], axis=1
    ).astype(np.float32)
    svo += b[None, :]
    svo = svo.reshape(B, OUT_F, D1)
    v = svo[:, :, 1:]
    biv = v[:, :, IU] * v[:, :, JU]
    out = np.zeros((B, OUT_F, MV_DIM), dtype=np.float32)
    out[:, :, SV_IDX] = svo
    out[:, :, BIV_IDX] = biv
    return out, res


def kernel(x, W, b):
    out, _ = run(x, W, b)
    return out


# revision 15
# speedup vs baseline: 1.1637x; 1.1637x over previous
"""CliffordLinearSimple on 8 Trainium2 NeuronCores.

Math (per reference):
    sv   = x[:, :, SV_IDX]                      # [B, IN_F, 9]  (scalar+vector slots)
    svo  = sv.reshape(B, IN_F*9) @ W.T + b      # [B, OUT_F*9]
    v    = svo.reshape(B, OUT_F, 9)[:, :, 1:]   # [B, OUT_F, 8]
    biv  = v[:, :, IU] * v[:, :, JU]            # [B, OUT_F, 28]
    out[..., SV_IDX] = svo; out[..., BIV_IDX] = biv; rest 0

Distribution: tensor-parallel over OUT_F (row-split W): core c owns out
slots [c*1152, (c+1)*1152).  The device does ONLY the GEMM
C[256, 1152] = svT.T @ W_c in bf16 (fp32 PSUM) and writes C back as
bf16; bias add, the 28 bivector products, and the scatter into the
[256, 1024, 256] multivector output all happen on the host in fp32
(exact, and bias in f32 is more accurate than the old bf16 device-side
bias matmul).

Schedule: k-outer over 18 groups of 4 k-tiles (K = 9216 = 72*128).
All six PSUM accumulators (2 batch tiles x 3 out-column tiles of
504+504+144) live for the whole kernel, so every group's DMA feeds the
same fixed compute:DMA ratio -- no front-loaded phase.  Three DMA
queues stream inputs concurrently (the old kernel's two HWDGE rings
capped input at ~334 GB/s while SWDGE sat idle until the output
drain): sync ring carries the n0 W chunks, scalar ring the n1 chunks,
and gpsimd/SWDGE carries svT + the narrow n2 chunks.  W is packed on
the host per (ring, group) into flat DRAM blocks so every W DMA is one
fully-sequential read; svT chunks are per-partition-contiguous slices.

Tail is just the final PSUM->SBUF bf16 casts (DVE) plus six ~100KB
output DMAs, instead of the old ~14us f32 compact-output + bivector
drain.  16 junk warm-up matmuls at the top keep the PE p-state ramp
off the critical path.
"""
import sys

if "/opt/trn_rl_repo" not in sys.path:
    sys.path.insert(0, "/opt/trn_rl_repo")

from contextlib import ExitStack

import ml_dtypes
import numpy as np

import concourse.bass as bass
import concourse.tile as tile
from concourse import bacc, mybir
from concourse.bass_utils import run_bass_kernel_spmd

ALG_DIM = 8
D1 = 9
MV_DIM = 256
B, IN_F, OUT_F = 256, 1024, 1024
POW2 = np.array([2 ** i for i in range(ALG_DIM)])
SV_IDX = np.concatenate([[0], POW2])
IU, JU = np.triu_indices(ALG_DIM, 1)
BIV_IDX = POW2[IU] + POW2[JU]
NCORES = 8
OF = OUT_F // NCORES          # 128 out features per core
N_CORE = OF * D1              # 1152 out slots per core
KT = IN_F * D1 // 128         # 72 k-tiles
BT = 2                        # batch tiles of 128

# out-column tiles (PSUM bank is 512 f32 wide) and k-groups.
# A DMA queue dispatches ~1 descriptor (= one partition line of
# G*NTILE*2 bytes) per ~35ns regardless of size, so queue throughput is
# line-size-bound: 4KB lines -> ~115 GB/s, 14KB -> ~260 GB/s ceiling.
# Small leading groups start the pipeline early (lines don't matter
# before the queues saturate); big trailing groups carry the bulk.
NTILES = (504, 504, 144)
NOFF = [sum(NTILES[:i]) for i in range(len(NTILES))]
KGRPS = [4, 6, 8, 12, 14, 14, 14]
KOFF = [sum(KGRPS[:i]) for i in range(len(KGRPS))]
NGRP = len(KGRPS)
# ring -> which n-tiles' W chunks it carries (rings: sync, scalar, gpsimd).
# The two HWDGE queues carry the wide n0/n1 W streams plus alternating
# svT chunks (SWDGE's ~93 GB/s ceiling would pace the whole pipeline if
# svT rode it -- every matmul gates on its svT chunk); SWDGE gets only
# the narrow n2 stream (2.65MB total, far under its ceiling).
RING_N = ([0], [1], [2])
WARM = 16


def build_core_program():
    assert sum(KGRPS) == KT and sum(NTILES) == N_CORE
    f32, bf16 = mybir.dt.float32, mybir.dt.bfloat16

    nc = bacc.Bacc("TRN2", target_bir_lowering=False, debug=False)
    svT_d = nc.dram_tensor("svT", [128, KT, B], bf16, kind="ExternalInput").ap()
    W_ds = [
        nc.dram_tensor(
            f"Wr{r}", [128 * sum(KGRPS) * sum(NTILES[n] for n in ns)], bf16,
            kind="ExternalInput",
        ).ap()
        if ns else None
        for r, ns in enumerate(RING_N)
    ]
    # [p, m*1152 + j] = C[m*128 + p, j]: keeps every partition's output
    # line contiguous (2304B) so the drain is one 128-descriptor DMA per
    # batch tile; the host undoes the interleave
    out_d = nc.dram_tensor("outc", [128, BT * N_CORE], bf16, kind="ExternalOutput").ap()

    with tile.TileContext(nc) as tc:
        with ExitStack() as ctx:
            const = ctx.enter_context(tc.tile_pool(name="const", bufs=1))
            wpools = [
                ctx.enter_context(tc.tile_pool(name=f"wp{r}", bufs=4))
                for r in range(len(RING_N))
            ]
            spool = ctx.enter_context(tc.tile_pool(name="spool", bufs=6))
            warmpool = ctx.enter_context(
                tc.tile_pool(name="warmpool", bufs=1, space="PSUM")
            )
            pspool = ctx.enter_context(
                tc.tile_pool(name="pspool", bufs=BT * len(NTILES), space="PSUM")
            )
            rings = [nc.sync, nc.scalar, nc.gpsimd]

            svT = const.tile([128, KT, B], bf16)

            # PE warm-up with no DMA deps: junk matmuls into a spare PSUM
            # bank release the HAM clock gate while the first W/svT chunks
            # are still in flight.
            warm_ps = warmpool.tile([128, 504], f32, name="warm", tag="warm")
            ones = const.tile([1, 128], bf16)
            nc.vector.memset(ones[:], 1.0)
            warm_rhs = const.tile([1, 504], bf16)
            nc.vector.memset(warm_rhs[:], 0.0)
            for _ in range(WARM):
                nc.tensor.matmul(
                    warm_ps[:], ones[:], warm_rhs[:],
                    start=True, stop=True, skip_group_check=True,
                )

            ps = {
                (m, n): pspool.tile([128, NTILES[n]], f32, name=f"ps{m}_{n}", tag="ps")
                for n in range(len(NTILES))
                for m in range(BT)
            }

            woff = [0] * len(RING_N)
            for g, G in enumerate(KGRPS):
                k0 = KOFF[g]
                # svT chunk g: per-partition contiguous 512*G-byte slice
                rings[g % 2].dma_start(svT[:, k0:k0 + G, :], svT_d[:, k0:k0 + G, :])
                wts = []
                for r, ns in enumerate(RING_N):
                    if not ns:
                        wts.append(None)
                        continue
                    cols = G * sum(NTILES[n] for n in ns)
                    wt = wpools[r].tile([128, cols], bf16, name=f"wt{r}", tag=f"wt{r}")
                    sz = 128 * cols
                    rings[r].dma_start(
                        wt[:],
                        W_ds[r][woff[r]:woff[r] + sz].rearrange("(p c) -> p c", p=128),
                    )
                    woff[r] += sz
                    wts.append(wt)
                last = g == NGRP - 1
                # ordinary groups: m inner per n-tile.  Final group: m OUTER,
                # so bank (0, *) all stop half a group early and their casts +
                # output DMA (whose ~128-descriptor dispatch costs ~4.5us on a
                # queue) overlap the m=1 matmuls instead of serializing after.
                for m_outer in range(BT if last else 1):
                    for r, ns in enumerate(RING_N):
                        coff = 0
                        for n in ns:
                            nt = NTILES[n]
                            for m in ([m_outer] if last else range(BT)):
                                for kl in range(G):
                                    nc.tensor.matmul(
                                        ps[(m, n)][:],
                                        svT[:, k0 + kl, m * 128:(m + 1) * 128],
                                        wts[r][:, coff + kl * nt:coff + (kl + 1) * nt],
                                        start=(g == 0 and kl == 0),
                                        stop=(last and kl == G - 1),
                                    )
                            coff += G * nt
                    if last:
                        m = m_outer
                        # drain batch-tile m: cast the three banks into one
                        # [128, 1152] staging tile (2304B lines -> a single
                        # 128-descriptor output DMA), split across both
                        # HWDGE queues for the final tile
                        st = spool.tile([128, N_CORE], bf16, name=f"st{m}", tag="st")
                        for n in range(len(NTILES)):
                            nc.vector.tensor_copy(
                                st[:, NOFF[n]:NOFF[n] + NTILES[n]], ps[(m, n)][:]
                            )
                        if m == 0:
                            rings[0].dma_start(out_d[:, :N_CORE], st[:])
                        else:
                            h = N_CORE // 2
                            rings[0].dma_start(
                                out_d[:, N_CORE:N_CORE + h], st[:, :h]
                            )
                            rings[1].dma_start(
                                out_d[:, N_CORE + h:], st[:, h:]
                            )

    nc.finalize()
    return nc


_PROGRAM = None


def _get_program():
    global _PROGRAM
    if _PROGRAM is None:
        _PROGRAM = build_core_program()
    return _PROGRAM


def _prep_inputs(x, W, b):
    bf16 = ml_dtypes.bfloat16
    # svT[p, kt, m] = sv[m, kt*128 + p], sv = x[:, :, SV_IDX] flattened
    sv = np.ascontiguousarray(x[:, :, SV_IDX]).reshape(B, IN_F * D1)
    svT = np.ascontiguousarray(sv.reshape(B, KT, 128).transpose(2, 1, 0)).astype(bf16)

    Wb = W.astype(bf16).reshape(NCORES, N_CORE, KT * 128)
    in_maps = []
    for c in range(NCORES):
        m = {"svT": svT}
        for r, ns in enumerate(RING_N):
            if not ns:
                continue
            parts = []
            for g, G in enumerate(KGRPS):
                k0 = KOFF[g]
                for n in ns:
                    # [p, kl, col] block, raveled: matches the device-side
                    # [128, G*NTILE] tile slice layout
                    blk = Wb[c, NOFF[n]:NOFF[n] + NTILES[n],
                             k0 * 128:(k0 + G) * 128]              # [col, G*128]
                    blk = blk.reshape(NTILES[n], G, 128).transpose(2, 1, 0)
                    parts.append(np.ascontiguousarray(blk).ravel())
            m[f"Wr{r}"] = np.concatenate(parts)
        in_maps.append(m)
    return in_maps


def run(x, W, b, trace=False):
    x = np.asarray(x, dtype=np.float32)
    W = np.asarray(W, dtype=np.float32)
    b = np.asarray(b, dtype=np.float32)
    in_maps = _prep_inputs(x, W, b)
    nc = _get_program()
    res = None
    for attempt in range(3):
        try:
            res = run_bass_kernel_spmd(
                nc, in_maps, core_ids=list(range(NCORES)), trace=trace
            )
            break
        except Exception:
            if attempt == 2:
                raise
            import time as _time
            _time.sleep(5)
    # host-side epilogue in f32: de-interleave [p, m, j] -> [m*128+p, j],
    # then bias, bivector products, scatter
    svo = np.concatenate(
        [
            np.asarray(res.results[c]["outc"])
            .reshape(128, BT, N_CORE)
            .transpose(1, 0, 2)
            .reshape(B, N_CORE)
            for c in range(NCORES)
        ],
        axis=1,
    ).astype(np.float32)
    svo += b[None, :]
    svo = svo.reshape(B, OUT_F, D1)
    v = svo[:, :, 1:]
    biv = v[:, :, IU] * v[:, :, JU]
    out = np.zeros((B, OUT_F, MV_DIM), dtype=np.float32)
    out[:, :, SV_IDX] = svo
    out[:, :, BIV_IDX] = biv
    return out, res


def kernel(x, W, b):
    out, _ = run(x, W, b)
    return out
